# revision 32
# baseline (speedup 1.0000x reference)
"""AdaptiveSegmenter Trainium2 kernel (8 NeuronCores, pure data parallel).

Per core: one batch row.  Pipeline (all on device):
  1. Boundary MLP in exact fp32 (decision margins go down to ~4e-7, so the
     boundary path cannot tolerate float32r's ~1e-4 rounding):
     h1 = gelu(x@W1.T+b1); l01d = [l0, l1, l1-l0] = h1@[W2;w2d].T + b.
  2. Flag pipeline in "KS layout" [128, 32] (t = 32p+f+1): boundary flags,
     reset/emit, cumsum (tensor_tensor_scan + strict-lower-tri matmul),
     scatter destinations.
  3. Segmented LSTM as a 33-step chunk-parallel scan: lanes k=0..127 process
     t = 32k+1+s (s in [0,32)); extra step -1 covers t = 32k.  Forced
     boundaries every 32 steps make lanes independent; data-dependent resets
     are applied via per-lane masks.  Gates G.T [lanes, 4D] accumulate in
     PSUM: bias (K=1 ones trick) + X-part + H-part, float32r (full PE rate
     at N=512).  The hidden state is re-transposed each step by a fused
     mask-diag matmul (h.T @ diag(1-r)).
  4. Emitted hidden states + boundary time indices scattered to DRAM by
     row-granular indirect DMA with OOB-skip.  bidx extracted from a
     [4096, 128] scatter staging buffer.
Host side: only layout prep (transposes/stacking) and unshard.
"""
import os
import numpy as np
from contextlib import ExitStack

import concourse.bass as bass
import concourse.bacc as bacc
import concourse.tile as tile
from concourse import mybir
from concourse.bass import IndirectOffsetOnAxis

B, T, D = 8, 4096, 512
H1 = 256          # D // 2
G4 = 2048         # 4 * D
NS = 32           # steps per lane (s = 0..31); step -1 extra
F32 = mybir.dt.float32
F32R = mybir.dt.float32r
I32 = mybir.dt.int32
AF = mybir.ActivationFunctionType
OP = mybir.AluOpType

EXPLICIT_ZERO_FILL = bool(int(os.environ.get("KERNEL_ZERO_FILL", "1")))


def build_nc():
    nc = bacc.Bacc()
    # ---- parameters (per-core shard = one batch row; weights replicated) ----
    xT = nc.declare_dram_parameter("xT", [D, T], F32, isOutput=False)
    wihT = nc.declare_dram_parameter("wihT", [D, G4], F32, isOutput=False)
    whhT = nc.declare_dram_parameter("whhT", [D, G4], F32, isOutput=False)
    w1T = nc.declare_dram_parameter("w1T", [D, H1], F32, isOutput=False)
    w23T = nc.declare_dram_parameter("w23T", [H1, 65], F32, isOutput=False)
    b1p = nc.declare_dram_parameter("b1p", [2, 128, 1], F32, isOutput=False)
    bias3 = nc.declare_dram_parameter("bias3", [1, 65], F32, isOutput=False)
    brow = nc.declare_dram_parameter("brow", [1, G4], F32, isOutput=False)
    lstrict = nc.declare_dram_parameter("lstrict", [128, 128], F32, isOutput=False)
    sshift = nc.declare_dram_parameter("sshift", [128, 128], F32, isOutput=False)
    ident = nc.declare_dram_parameter("ident", [128, 128], F32, isOutput=False)
    forced = nc.declare_dram_parameter("forced", [128, NS], F32, isOutput=False)
    tvals = nc.declare_dram_parameter("tvals", [128, NS], F32, isOutput=False)

    padded = nc.declare_dram_parameter("padded", [T, D], F32, isOutput=True)
    bidx_o = nc.declare_dram_parameter("bidx_o", [32, 128], I32, isOutput=True)
    logits = nc.declare_dram_parameter("logits", [2, T], F32, isOutput=True)

    B1 = nc.dram_tensor("B1", [T, 32], F32)    # bidx scatter staging

    with tile.TileContext(nc) as tc, ExitStack() as top:
        sb = top.enter_context(tc.tile_pool(name="sb", bufs=1))

        # ---------- small constants ----------
        b1t = []
        for hh in range(2):
            bt = sb.tile([128, 1], F32, tag=f"b1_{hh}")
            nc.sync.dma_start(bt[:], b1p[hh])
            b1t.append(bt)
        Ls = sb.tile([128, 128], F32, tag="Ls")
        Ss = sb.tile([128, 128], F32, tag="Ss")
        Id = sb.tile([128, 128], F32, tag="Id")
        fo = sb.tile([128, NS], F32, tag="fo")
        tv = sb.tile([128, NS], F32, tag="tv")
        nc.sync.dma_start(Ls[:], lstrict[:])
        nc.sync.dma_start(Ss[:], sshift[:])
        nc.sync.dma_start(Id[:], ident[:])
        nc.sync.dma_start(fo[:], forced[:])
        nc.sync.dma_start(tv[:], tvals[:])
        ones128 = sb.tile([128, 128], F32, tag="ones128")
        nc.vector.memset(ones128[:], 1.0)
        neg1 = sb.tile([128, 128], F32, tag="neg1")
        nc.vector.memset(neg1[:], -1.0)
        zero_t = sb.tile([128, 512], F32, tag="zero_t")
        nc.vector.memset(zero_t[:], 0.0)

        # ---------- flag tiles (persistent; filled in phase 2) ----------
        dk = sb.tile([128, NS], F32, tag="dk")
        isb = sb.tile([128, NS], F32, tag="isb")
        notr = sb.tile([128, NS], F32, tag="notr")
        emit = sb.tile([128, NS], F32, tag="emit")
        cums = sb.tile([128, NS], F32, tag="cums")
        dest_f = sb.tile([128, NS], F32, tag="dest_f")
        dest_i = sb.tile([128, NS], I32, tag="dest_i")
        emit_i = sb.tile([128, NS], I32, tag="emit_i")
        big = sb.tile([128, NS], F32, tag="bigt")
        zero32 = sb.tile([128, NS], F32, tag="zero32")
        r0col = sb.tile([128, 1], F32, tag="r0col")
        offs = sb.tile([128, 1], F32, tag="offs")
        cnt = sb.tile([1, 1], F32, tag="cnt")
        zerocol = sb.tile([128, 1], F32, tag="zerocol")
        nc.vector.memset(zerocol[:], 0.0)
        nc.vector.memset(zero32[:], 0.0)
        nc.vector.memset(big[:], float(T))

        B1r3 = B1[:].rearrange("(a p) f -> a p f", p=128)
        scanp = top.enter_context(tc.tile_pool(name="scanp", bufs=1))

        # ================= Phase 0/1: x load (fp32) + MLP (fp32) ============
        xf_scope = ExitStack()
        mlp_scope = ExitStack()
        xfpool = xf_scope.enter_context(tc.tile_pool(name="xfpool", bufs=1))
        lpool = mlp_scope.enter_context(tc.tile_pool(name="lpool", bufs=1))
        x3 = xT[:].rearrange("(kt p) t -> kt p t", p=128)
        xf = []
        for kt in range(4):
            xft = xfpool.tile([128, T], F32, tag=f"xf{kt}")
            nc.sync.dma_start(xft[:], x3[kt])
            xf.append(xft)
        l01d = lpool.tile([65, T + 16], F32, tag="l01d")
        nc.vector.memset(l01d[64:65, T:T + 16], -1.0)


        with tc.tile_pool(name="mlpw", bufs=1) as mlpw, \
             tc.tile_pool(name="pmlp", bufs=2, space="PSUM") as pmlp, \
             tc.tile_pool(name="pmlp2", bufs=2, space="PSUM") as pmlp2, \
             tc.tile_pool(name="h1pool", bufs=4) as h1pool:
            w1r = []
            w13 = w1T[:].rearrange("(kt p) h -> kt p h", p=128)
            for kt in range(4):
                w1t_ = mlpw.tile([128, H1], F32, tag=f"w1{kt}")
                nc.sync.dma_start(w1t_[:], w13[kt])
                w1r.append(w1t_)
            w23r = []
            w233 = w23T[:].rearrange("(ht p) g -> ht p g", p=128)
            for ht in range(2):
                wt = mlpw.tile([128, 65], F32, tag=f"w23{ht}")
                nc.sync.dma_start(wt[:], w233[ht])
                w23r.append(wt)
            bias3f = mlpw.tile([1, 65], F32, tag="bias3f")
            nc.sync.dma_start(bias3f[:], bias3[:])
            ones_r = mlpw.tile([1, 512], F32, tag="ones_r")
            nc.vector.memset(ones_r[:], 1.0)

            # B1 prefill with -1 (bidx default); optional padded zero-fill
            for i in range(32):
                nc.sync.dma_start(B1r3[i], neg1[:, 0:32])
            if EXPLICIT_ZERO_FILL:
                pad3 = padded[:].rearrange("(a p) d -> a p d", p=128)
                for i in range(32):
                    nc.sync.dma_start(pad3[i], zero_t[:])

            # x -> f32r for the scan via SWDGE cast DMA (after MLP's HWDGE loads)
            xr = []
            for kt in range(4):
                xrt = scanp.tile([128, T + 32], F32R, tag=f"xr{kt}")
                nc.gpsimd.dma_start(xrt[:, 0:T], x3[kt])
                nc.vector.tensor_copy(xrt[:, T:T + 32], zero_t[:, 0:32])
                xr.append(xrt)

            for tci in range(8):
                tsl = slice(512 * tci, 512 * tci + 512)
                h1c = []
                for hh in range(2):
                    ps1 = pmlp.tile([128, 512], F32, tag="ps1")
                    for kt in range(4):
                        nc.tensor.matmul(ps1[:], w1r[kt][:, 128 * hh:128 * hh + 128],
                                         xf[kt][:, tsl], start=(kt == 0), stop=(kt == 3))
                    hc = h1pool.tile([128, 512], F32, tag="h1c")
                    nc.scalar.activation(hc[:], ps1[:], AF.Gelu, bias=b1t[hh][:])
                    h1c.append(hc)
                ps2 = pmlp2.tile([65, 512], F32, tag="ps2")
                nc.tensor.matmul(ps2[:], bias3f[:], ones_r[:], start=True, stop=False)
                for hh in range(2):
                    nc.tensor.matmul(ps2[:], w23r[hh][:], h1c[hh][:],
                                     start=False, stop=(hh == 1))
                nc.vector.tensor_copy(l01d[:, tsl], ps2[:])
        nc.sync.dma_start(logits[:], l01d[0:64:32, 0:T])

        # ================= Phase 2: flags — DVE prep (no PSUM) =============
        nc.sync.dma_start(dk[:], l01d[64:65, 1:T + 1])
        nc.vector.tensor_scalar(dk[:], dk[:], 0.0, None, op0=OP.is_gt)
        nc.vector.tensor_tensor(isb[:], dk[:], fo[:], op=OP.max)
        d0f = sb.tile([1, 1], F32, tag="d0f")
        nc.vector.tensor_scalar(d0f[:], l01d[64:65, 0:1], 0.0, None, op0=OP.is_gt)
        mlp_scope.close()
        xf_scope.close()

        # ================= Phase 2b: W loads (f32r, SWDGE cast) =============
        wpool = top.enter_context(tc.tile_pool(name="wpool", bufs=1))
        biasrep = wpool.tile([128, G4], F32, tag="biasrep")
        onesv = wpool.tile([1, 128], F32, tag="onesv")
        browf = wpool.tile([1, G4], F32, tag="browf")
        nc.sync.dma_start(browf[:], brow[:])
        nc.vector.tensor_copy(onesv[:], ones128[0:1, 0:128])
        wih_r, whh_r = [], []
        for src, dst_list, nm in ((wihT, wih_r, "wih"), (whhT, whh_r, "whh")):
            w3 = src[:].rearrange("(kt p) g -> kt p g", p=128)
            for kt in range(4):
                wr = wpool.tile([128, G4], F32R, tag=f"{nm}{kt}")
                nc.gpsimd.dma_start(wr[:], w3[kt])
                dst_list.append(wr)

        # ================= Phase 3: scan =================
        with tc.tile_pool(name="pg", bufs=3, space="PSUM") as pg, \
             tc.tile_pool(name="pt", bufs=2, space="PSUM") as pt, \
             tc.tile_pool(name="acts", bufs=12) as acts, \
             tc.tile_pool(name="cpool", bufs=3) as cpool, \
             tc.tile_pool(name="hpool", bufs=4) as hpool, \
             tc.tile_pool(name="htpool", bufs=8) as htpool, \
             tc.tile_pool(name="tvpool", bufs=3) as tvpool:

            c_prev = cpool.tile([128, 512], F32, tag="c")
            nc.vector.memset(c_prev[:], 0.0)
            HT_prev = None
            breg = nc.gpsimd.to_reg(T - 1)

            def emit_x(gAt, gBt, st, stop):
                xsl = slice(1 + st, 1 + st + 32 * 128, 32)
                for half, gt in ((0, gAt), (1, gBt)):
                    for nch in range(2):
                        gsl = gt[:, 512 * nch:512 * nch + 512]
                        gw = slice(1024 * half + 512 * nch,
                                   1024 * half + 512 * nch + 512)
                        for kt in range(4):
                            nc.tensor.matmul(gsl, xr[kt][:, xsl], wih_r[kt][:, gw],
                                             start=(kt == 0), stop=(stop and kt == 3))

            # prefetch X for step -1 (PE busy while flags compute on DVE)
            gA = pg.tile([128, 1024], F32, tag="g")
            gB = pg.tile([128, 1024], F32, tag="g")
            emit_x(gA, gB, -1, True)

            # bias broadcast (K=1 matmul trick) into biasrep
            for ch in range(4):
                bps = pt.tile([128, 512], F32, tag="pt")
                nc.tensor.matmul(bps[:], onesv[:], browf[:, 512 * ch:512 * ch + 512],
                                 start=True, stop=True)
                nc.vector.tensor_copy(biasrep[:, 512 * ch:512 * ch + 512], bps[:])

            # flags: PSUM-needing pieces from the pt pool
            r0p = pt.tile([128, 1], F32, tag="pt")
            nc.tensor.matmul(r0p[:], Ss[:], isb[:, NS - 1:NS], start=True, stop=True)
            nc.vector.tensor_copy(r0col[:], r0p[:])
            nc.vector.tensor_copy(r0col[0:1, 0:1], d0f[:])
            nc.vector.tensor_scalar(notr[:, 0:1], r0col[:], -1.0, -1.0,
                                    op0=OP.mult, op1=OP.subtract)
            nc.vector.tensor_scalar(notr[:, 1:NS], isb[:, 0:NS - 1], -1.0, -1.0,
                                    op0=OP.mult, op1=OP.subtract)
            nc.vector.tensor_tensor(emit[:], isb[:], notr[:], op=OP.mult)
            nc.vector.tensor_tensor_scan(cums[:], emit[:], zero32[:], 0.0,
                                         op0=OP.add, op1=OP.add)
            offp = pt.tile([128, 1], F32, tag="pt")
            nc.tensor.matmul(offp[:], Ls[:], cums[:, NS - 1:NS], start=True, stop=True)
            nc.vector.tensor_copy(offs[:], offp[:])
            nc.vector.tensor_scalar(dest_f[:], cums[:], offs[:], -1.0,
                                    op0=OP.add, op1=OP.add)
            nc.vector.tensor_copy(emit_i[:], emit[:])
            nc.vector.copy_predicated(big[:], emit_i[:], dest_f[:])
            nc.vector.tensor_copy(dest_i[:], big[:])
            cntp = pt.tile([1, 1], F32, tag="pt")
            nc.tensor.matmul(cntp[:], ones128[:, 0:1], cums[:, NS - 1:NS],
                             start=True, stop=True)
            nc.vector.tensor_copy(cnt[:], cntp[:])

            for s in range(-1, NS):
                nr = zerocol if s == -1 else notr[:, s:s + 1]
                if s >= 0:
                    for half, gt in ((0, gA), (1, gB)):
                        for nch in range(2):
                            gsl = gt[:, 512 * nch:512 * nch + 512]
                            gw = slice(1024 * half + 512 * nch,
                                       1024 * half + 512 * nch + 512)
                            for kt in range(4):
                                nc.tensor.matmul(gsl, HT_prev[kt][:], whh_r[kt][:, gw],
                                                 start=False, stop=(kt == 3))
                # prefetch next step's X while this step's cell runs
                if s < NS - 1:
                    gA2 = pg.tile([128, 1024], F32, tag="g")
                    gB2 = pg.tile([128, 1024], F32, tag="g")
                    emit_x(gA2, gB2, s + 1, False)
                # cell: bias-add then activations, half-split along hidden
                # so downstream transposes/H-matmuls unblock incrementally
                gi = acts.tile([128, 512], F32, tag="act")
                gf = acts.tile([128, 512], F32, tag="act")
                gg = acts.tile([128, 512], F32, tag="act")
                go_ = acts.tile([128, 512], F32, tag="act")
                c_new = cpool.tile([128, 512], F32, tag="c")
                pmul = acts.tile([128, 512], F32, tag="act")
                tc_t = acts.tile([128, 512], F32, tag="act")
                for hh2 in range(2):
                    hsl = slice(256 * hh2, 256 * hh2 + 256)
                    hslB = slice(512 + 256 * hh2, 512 + 256 * hh2 + 256)
                    nc.vector.tensor_add(gg[:, hsl], gB[:, hsl],
                                         biasrep[:, 1024 + 256 * hh2:1280 + 256 * hh2])
                    nc.vector.tensor_add(gi[:, hsl], gA[:, hsl],
                                         biasrep[:, 256 * hh2:256 + 256 * hh2])
                    nc.vector.tensor_add(gf[:, hsl], gA[:, hslB],
                                         biasrep[:, 512 + 256 * hh2:768 + 256 * hh2])
                    nc.vector.tensor_add(go_[:, hsl], gB[:, hslB],
                                         biasrep[:, 1536 + 256 * hh2:1792 + 256 * hh2])
                    nc.scalar.activation(gg[:, hsl], gg[:, hsl], AF.Tanh)
                    nc.scalar.activation(gi[:, hsl], gi[:, hsl], AF.Sigmoid)
                    nc.scalar.activation(gf[:, hsl], gf[:, hsl], AF.Sigmoid)
                    nc.scalar.activation(go_[:, hsl], go_[:, hsl], AF.Sigmoid)
                    nc.vector.tensor_mul(pmul[:, hsl], gi[:, hsl], gg[:, hsl])
                    nc.vector.scalar_tensor_tensor(c_new[:, hsl], c_prev[:, hsl],
                                                   nr[:], gf[:, hsl],
                                                   op0=OP.mult, op1=OP.mult)
                    nc.vector.tensor_add(c_new[:, hsl], c_new[:, hsl], pmul[:, hsl])
                    nc.scalar.activation(tc_t[:, hsl], c_new[:, hsl], AF.Tanh)
                sig_o = go_

                if s >= 0:
                    h_t = hpool.tile([128, 512], F32, tag="h")
                    nc.gpsimd.tensor_mul(h_t[:], sig_o[:], tc_t[:])
                    nc.gpsimd.indirect_dma_start(
                        out=padded[:],
                        out_offset=IndirectOffsetOnAxis(ap=dest_i[:, s:s + 1], axis=0),
                        in_=h_t[:], in_offset=None,
                        bounds_check=breg, oob_is_err=False,
                    )
                    tvb = tvpool.tile([128, 32], F32, tag="tvb")
                    nc.scalar.activation(tvb[:], ones128[:, 0:32], AF.Copy,
                                         scale=tv[:, s:s + 1])
                    nc.gpsimd.indirect_dma_start(
                        out=B1[:],
                        out_offset=IndirectOffsetOnAxis(ap=dest_i[:, s:s + 1], axis=0),
                        in_=tvb[:], in_offset=None,
                        bounds_check=breg, oob_is_err=False,
                    )

                if s < NS - 1:
                    # next-step stationary: transpose of (sig_o * notr[s+1] * tanh_c)
                    h_m = hpool.tile([128, 512], F32, tag="hm")
                    for i in range(4):
                        csl = slice(128 * i, 128 * i + 128)
                        nc.vector.scalar_tensor_tensor(h_m[:, csl], sig_o[:, csl],
                                                       notr[:, s + 1:s + 2],
                                                       tc_t[:, csl],
                                                       op0=OP.mult, op1=OP.mult)
                    ptile = pt.tile([128, 512], F32, tag="pt")
                    for i in range(4):
                        nc.tensor.matmul(ptile[:, 128 * i:128 * i + 128],
                                         h_m[:, 128 * i:128 * i + 128], Id[:],
                                         is_transpose=True,
                                         start=(i == 0), stop=(i == 3))
                    HT = []
                    for i in range(4):
                        htt = htpool.tile([128, 128], F32R, tag="ht")
                        if i % 2 == 0:
                            nc.vector.tensor_copy(htt[:], ptile[:, 128 * i:128 * i + 128])
                        else:
                            nc.scalar.copy(htt[:], ptile[:, 128 * i:128 * i + 128])
                        HT.append(htt)
                    HT_prev = HT
                c_prev = c_new
                if s < NS - 1:
                    gA, gB = gA2, gB2

        # ================= Phase 4: bidx extraction =================
        with tc.tile_pool(name="bex", bufs=1) as bex, \
             tc.tile_pool(name="pbx", bufs=1, space="PSUM") as pbx:
            ext = bex.tile([128, 1024], F32, tag="ext")
            for bb in range(32):
                eng = nc.sync if bb % 2 == 0 else nc.scalar
                eng.dma_start(ext[:, 32 * bb:32 * bb + 32], B1r3[bb])
            bf = bex.tile([128, 32], F32, tag="bf")
            nc.vector.tensor_copy(bf[:], ext[:, 0:1024:32])
            bp = pbx.tile([32, 128], F32, tag="bp")
            nc.tensor.matmul(bp[:], bf[:], Id[:], is_transpose=True,
                             start=True, stop=True)
            bout = bex.tile([32, 128], F32, tag="bout")
            nc.vector.tensor_copy(bout[:], bp[:])
            fb = bex.tile([1, 1], F32, tag="fb")
            nc.vector.tensor_scalar(fb[:], cnt[:], 0.0, None, op0=OP.is_equal)
            fbi = bex.tile([1, 1], I32, tag="fbi")
            nc.vector.tensor_copy(fbi[:], fb[:])
            fbv = bex.tile([1, 1], F32, tag="fbv")
            nc.vector.memset(fbv[:], float(T - 1))
            nc.vector.copy_predicated(bout[0:1, 0:1], fbi[:], fbv[:])
            bi = bex.tile([32, 128], I32, tag="bi")
            nc.vector.tensor_copy(bi[:], bout[:])
            nc.sync.dma_start(bidx_o[:], bi[:])

    nc.compile()
    return nc


def _w23_65(W2):
    w = np.zeros((H1, 65), np.float32)
    w[:, 0] = W2[0]
    w[:, 32] = W2[1]
    w[:, 64] = W2[1] - W2[0]
    return w


def _bias65(b2):
    b = np.zeros((1, 65), np.float32)
    b[0, 0] = b2[0]
    b[0, 32] = b2[1]
    b[0, 64] = b2[1] - b2[0]
    return b


def make_host_inputs(x, W1, b1, W2, b2, W_ih, W_hh, b_ih, b_hh):
    """Returns per-core list of input dicts (host-side layout prep only)."""
    lanes = np.arange(128)
    lstrict = (lanes[:, None] < lanes[None, :]).astype(np.float32)      # L[q,p]=1 if q<p
    sshift = (lanes[:, None] == lanes[None, :] - 1).astype(np.float32)  # S[q,p]=1 if q==p-1
    ident = np.eye(128, dtype=np.float32)
    forced = np.zeros((128, NS), np.float32)
    forced[:, 31] = 1.0
    forced[127, 31] = 0.0
    forced[127, 30] = 1.0
    tvals = (32 * lanes[:, None] + 1 + np.arange(NS)[None, :]).astype(np.float32)
    common = {
        "wihT": np.ascontiguousarray(W_ih.T),
        "whhT": np.ascontiguousarray(W_hh.T),
        "w1T": np.ascontiguousarray(W1.T),
        "w23T": _w23_65(W2),
        "b1p": b1.reshape(2, 128, 1).astype(np.float32),
        "bias3": _bias65(b2),
        "brow": (b_ih + b_hh).reshape(1, G4).astype(np.float32),
        "lstrict": lstrict, "sshift": sshift, "ident": ident,
        "forced": forced, "tvals": tvals,
    }
    per_core = []
    for b in range(B):
        m = dict(common)
        m["xT"] = np.ascontiguousarray(x[b].T)
        per_core.append(m)
    return per_core


_NC_CACHE = {}


def kernel(x, W1, b1, W2, b2, W_ih, W_hh, b_ih, b_hh):
    from concourse.bass_utils import run_bass_kernel_spmd

    x = np.asarray(x, np.float32)
    args = [np.asarray(a, np.float32) for a in (W1, b1, W2, b2, W_ih, W_hh, b_ih, b_hh)]
    if "nc" not in _NC_CACHE:
        _NC_CACHE["nc"] = build_nc()
    nc = _NC_CACHE["nc"]
    in_maps = make_host_inputs(x, *args)
    res = run_bass_kernel_spmd(nc, in_maps, core_ids=list(range(B)))
    padded = np.stack([res.results[c]["padded"] for c in range(B)])
    bidx = np.stack([res.results[c]["bidx_o"].reshape(T) for c in range(B)])
    logits = np.stack([np.ascontiguousarray(res.results[c]["logits"].T)
                       for c in range(B)])
    return padded, bidx, logits


# revision 33
# speedup vs baseline: 1.0181x; 1.0181x over previous
"""AdaptiveSegmenter Trainium2 kernel (8 NeuronCores, pure data parallel).

Per core: one batch row.  Pipeline (all on device):
  1. Boundary MLP in exact fp32 (decision margins go down to ~4e-7, so the
     boundary path cannot tolerate float32r's ~1e-4 rounding):
     h1 = gelu(x@W1.T+b1); l01d = [l0, l1, l1-l0] = h1@[W2;w2d].T + b.
  2. Flag pipeline in "KS layout" [128, 32] (t = 32p+f+1): boundary flags,
     reset/emit, cumsum (tensor_tensor_scan + strict-lower-tri matmul),
     scatter destinations.
  3. Segmented LSTM as a 33-step chunk-parallel scan: lanes k=0..127 process
     t = 32k+1+s (s in [0,32)); extra step -1 covers t = 32k.  Forced
     boundaries every 32 steps make lanes independent; data-dependent resets
     are applied via per-lane masks.  Gates G.T [lanes, 4D] accumulate in
     PSUM: bias (K=1 ones trick) + X-part + H-part, float32r (full PE rate
     at N=512).  The hidden state is re-transposed each step by a fused
     mask-diag matmul (h.T @ diag(1-r)).
  4. Emitted hidden states + boundary time indices scattered to DRAM by
     row-granular indirect DMA with OOB-skip.  bidx extracted from a
     [4096, 128] scatter staging buffer.
Host side: only layout prep (transposes/stacking) and unshard.
"""
import os
import numpy as np
from contextlib import ExitStack

import concourse.bass as bass
import concourse.bacc as bacc
import concourse.tile as tile
from concourse import mybir
from concourse.bass import IndirectOffsetOnAxis

B, T, D = 8, 4096, 512
H1 = 256          # D // 2
G4 = 2048         # 4 * D
NS = 32           # steps per lane (s = 0..31); step -1 extra
F32 = mybir.dt.float32
F32R = mybir.dt.float32r
I32 = mybir.dt.int32
AF = mybir.ActivationFunctionType
OP = mybir.AluOpType

EXPLICIT_ZERO_FILL = bool(int(os.environ.get("KERNEL_ZERO_FILL", "1")))


def build_nc():
    nc = bacc.Bacc()
    # ---- parameters (per-core shard = one batch row; weights replicated) ----
    xT = nc.declare_dram_parameter("xT", [D, T], F32, isOutput=False)
    wihT = nc.declare_dram_parameter("wihT", [D, G4], F32, isOutput=False)
    whhT = nc.declare_dram_parameter("whhT", [D, G4], F32, isOutput=False)
    w1T = nc.declare_dram_parameter("w1T", [D, H1], F32, isOutput=False)
    w23T = nc.declare_dram_parameter("w23T", [H1, 65], F32, isOutput=False)
    b1p = nc.declare_dram_parameter("b1p", [2, 128, 1], F32, isOutput=False)
    bias3 = nc.declare_dram_parameter("bias3", [1, 65], F32, isOutput=False)
    brow = nc.declare_dram_parameter("brow", [1, G4], F32, isOutput=False)
    lstrict = nc.declare_dram_parameter("lstrict", [128, 128], F32, isOutput=False)
    sshift = nc.declare_dram_parameter("sshift", [128, 128], F32, isOutput=False)
    ident = nc.declare_dram_parameter("ident", [128, 128], F32, isOutput=False)
    forced = nc.declare_dram_parameter("forced", [128, NS], F32, isOutput=False)
    tvals = nc.declare_dram_parameter("tvals", [128, NS], F32, isOutput=False)

    padded = nc.declare_dram_parameter("padded", [T, D], F32, isOutput=True)
    bidx_o = nc.declare_dram_parameter("bidx_o", [32, 128], I32, isOutput=True)
    logits = nc.declare_dram_parameter("logits", [2, T], F32, isOutput=True)

    B1 = nc.dram_tensor("B1", [T, 32], F32)    # bidx scatter staging

    with tile.TileContext(nc) as tc, ExitStack() as top:
        sb = top.enter_context(tc.tile_pool(name="sb", bufs=1))

        # ---------- small constants ----------
        b1t = []
        for hh in range(2):
            bt = sb.tile([128, 1], F32, tag=f"b1_{hh}")
            nc.sync.dma_start(bt[:], b1p[hh])
            b1t.append(bt)
        Ls = sb.tile([128, 128], F32, tag="Ls")
        Ss = sb.tile([128, 128], F32, tag="Ss")
        Id = sb.tile([128, 128], F32, tag="Id")
        fo = sb.tile([128, NS], F32, tag="fo")
        tv = sb.tile([128, NS], F32, tag="tv")
        nc.sync.dma_start(Ls[:], lstrict[:])
        nc.sync.dma_start(Ss[:], sshift[:])
        nc.sync.dma_start(Id[:], ident[:])
        nc.sync.dma_start(fo[:], forced[:])
        nc.sync.dma_start(tv[:], tvals[:])
        ones128 = sb.tile([128, 128], F32, tag="ones128")
        nc.vector.memset(ones128[:], 1.0)
        neg1 = sb.tile([128, 128], F32, tag="neg1")
        nc.vector.memset(neg1[:], -1.0)
        zero_t = sb.tile([128, 512], F32, tag="zero_t")
        nc.vector.memset(zero_t[:], 0.0)

        # ---------- flag tiles (persistent; filled in phase 2) ----------
        dk = sb.tile([128, NS], F32, tag="dk")
        isb = sb.tile([128, NS], F32, tag="isb")
        notr = sb.tile([128, NS], F32, tag="notr")
        emit = sb.tile([128, NS], F32, tag="emit")
        cums = sb.tile([128, NS], F32, tag="cums")
        dest_f = sb.tile([128, NS], F32, tag="dest_f")
        dest_i = sb.tile([128, NS], I32, tag="dest_i")
        emit_i = sb.tile([128, NS], I32, tag="emit_i")
        big = sb.tile([128, NS], F32, tag="bigt")
        zero32 = sb.tile([128, NS], F32, tag="zero32")
        r0col = sb.tile([128, 1], F32, tag="r0col")
        offs = sb.tile([128, 1], F32, tag="offs")
        cnt = sb.tile([1, 1], F32, tag="cnt")
        zerocol = sb.tile([128, 1], F32, tag="zerocol")
        nc.vector.memset(zerocol[:], 0.0)
        nc.vector.memset(zero32[:], 0.0)
        nc.vector.memset(big[:], float(T))

        B1r3 = B1[:].rearrange("(a p) f -> a p f", p=128)
        scanp = top.enter_context(tc.tile_pool(name="scanp", bufs=1))

        # ================= Phase 0/1: x load (fp32) + MLP (fp32) ============
        xf_scope = ExitStack()
        mlp_scope = ExitStack()
        xfpool = xf_scope.enter_context(tc.tile_pool(name="xfpool", bufs=1))
        lpool = mlp_scope.enter_context(tc.tile_pool(name="lpool", bufs=1))
        x3 = xT[:].rearrange("(kt p) t -> kt p t", p=128)
        xf = []
        for kt in range(4):
            xft = xfpool.tile([128, T], F32, tag=f"xf{kt}")
            nc.sync.dma_start(xft[:], x3[kt])
            xf.append(xft)
        l01d = lpool.tile([65, T + 16], F32, tag="l01d")
        nc.vector.memset(l01d[64:65, T:T + 16], -1.0)


        with tc.tile_pool(name="mlpw", bufs=1) as mlpw, \
             tc.tile_pool(name="pmlp", bufs=2, space="PSUM") as pmlp, \
             tc.tile_pool(name="pmlp2", bufs=2, space="PSUM") as pmlp2, \
             tc.tile_pool(name="h1pool", bufs=4) as h1pool:
            w1r = []
            w13 = w1T[:].rearrange("(kt p) h -> kt p h", p=128)
            for kt in range(4):
                w1t_ = mlpw.tile([128, H1], F32, tag=f"w1{kt}")
                nc.sync.dma_start(w1t_[:], w13[kt])
                w1r.append(w1t_)
            w23r = []
            w233 = w23T[:].rearrange("(ht p) g -> ht p g", p=128)
            for ht in range(2):
                wt = mlpw.tile([128, 65], F32, tag=f"w23{ht}")
                nc.sync.dma_start(wt[:], w233[ht])
                w23r.append(wt)
            bias3f = mlpw.tile([1, 65], F32, tag="bias3f")
            nc.sync.dma_start(bias3f[:], bias3[:])
            ones_r = mlpw.tile([1, 512], F32, tag="ones_r")
            nc.vector.memset(ones_r[:], 1.0)

            # B1 prefill with -1 (bidx default); optional padded zero-fill
            for i in range(32):
                nc.sync.dma_start(B1r3[i], neg1[:, 0:32])
            if EXPLICIT_ZERO_FILL:
                pad3 = padded[:].rearrange("(a p) d -> a p d", p=128)
                for i in range(32):
                    nc.sync.dma_start(pad3[i], zero_t[:])

            # x -> f32r for the scan via SWDGE cast DMA (after MLP's HWDGE loads)
            xr = []
            for kt in range(4):
                xrt = scanp.tile([128, T + 32], F32R, tag=f"xr{kt}")
                nc.gpsimd.dma_start(xrt[:, 0:T], x3[kt])
                nc.vector.tensor_copy(xrt[:, T:T + 32], zero_t[:, 0:32])
                xr.append(xrt)

            for tci in range(8):
                tsl = slice(512 * tci, 512 * tci + 512)
                h1c = []
                for hh in range(2):
                    ps1 = pmlp.tile([128, 512], F32, tag="ps1")
                    for kt in range(4):
                        nc.tensor.matmul(ps1[:], w1r[kt][:, 128 * hh:128 * hh + 128],
                                         xf[kt][:, tsl], start=(kt == 0), stop=(kt == 3))
                    hc = h1pool.tile([128, 512], F32, tag="h1c")
                    nc.scalar.activation(hc[:], ps1[:], AF.Gelu, bias=b1t[hh][:])
                    h1c.append(hc)
                ps2 = pmlp2.tile([65, 512], F32, tag="ps2")
                nc.tensor.matmul(ps2[:], bias3f[:], ones_r[:], start=True, stop=False)
                for hh in range(2):
                    nc.tensor.matmul(ps2[:], w23r[hh][:], h1c[hh][:],
                                     start=False, stop=(hh == 1))
                nc.vector.tensor_copy(l01d[:, tsl], ps2[:])
        nc.sync.dma_start(logits[:], l01d[0:64:32, 0:T])

        # ================= Phase 2: flags — DVE prep (no PSUM) =============
        nc.sync.dma_start(dk[:], l01d[64:65, 1:T + 1])
        nc.vector.tensor_scalar(dk[:], dk[:], 0.0, None, op0=OP.is_gt)
        nc.vector.tensor_tensor(isb[:], dk[:], fo[:], op=OP.max)
        d0f = sb.tile([1, 1], F32, tag="d0f")
        nc.vector.tensor_scalar(d0f[:], l01d[64:65, 0:1], 0.0, None, op0=OP.is_gt)
        mlp_scope.close()
        xf_scope.close()

        # ================= Phase 2b: W loads (f32r, SWDGE cast) =============
        wpool = top.enter_context(tc.tile_pool(name="wpool", bufs=1))
        biasrep = wpool.tile([128, G4], F32, tag="biasrep")
        onesv = wpool.tile([1, 128], F32, tag="onesv")
        browf = wpool.tile([1, G4], F32, tag="browf")
        nc.sync.dma_start(browf[:], brow[:])
        nc.vector.tensor_copy(onesv[:], ones128[0:1, 0:128])
        wih_r, whh_r = [], []
        for src, dst_list, nm in ((wihT, wih_r, "wih"), (whhT, whh_r, "whh")):
            w3 = src[:].rearrange("(kt p) g -> kt p g", p=128)
            for kt in range(4):
                wr = wpool.tile([128, G4], F32R, tag=f"{nm}{kt}")
                nc.gpsimd.dma_start(wr[:], w3[kt])
                dst_list.append(wr)

        # ================= Phase 3: scan =================
        with tc.tile_pool(name="pg", bufs=3, space="PSUM") as pg, \
             tc.tile_pool(name="pt", bufs=2, space="PSUM") as pt, \
             tc.tile_pool(name="acts", bufs=12) as acts, \
             tc.tile_pool(name="cpool", bufs=3) as cpool, \
             tc.tile_pool(name="hpool", bufs=4) as hpool, \
             tc.tile_pool(name="htpool", bufs=8) as htpool, \
             tc.tile_pool(name="tvpool", bufs=3) as tvpool:

            c_prev = cpool.tile([128, 512], F32, tag="c")
            nc.vector.memset(c_prev[:], 0.0)
            HT_prev = None
            breg = nc.gpsimd.to_reg(T - 1)

            def emit_x(gAt, gBt, st, stop):
                xsl = slice(1 + st, 1 + st + 32 * 128, 32)
                for half, gt in ((0, gAt), (1, gBt)):
                    for nch in range(2):
                        gsl = gt[:, 512 * nch:512 * nch + 512]
                        gw = slice(1024 * half + 512 * nch,
                                   1024 * half + 512 * nch + 512)
                        for kt in range(4):
                            nc.tensor.matmul(gsl, xr[kt][:, xsl], wih_r[kt][:, gw],
                                             start=(kt == 0), stop=(stop and kt == 3))

            # prefetch X for step -1 (PE busy while flags compute on DVE)
            gA = pg.tile([128, 1024], F32, tag="g")
            gB = pg.tile([128, 1024], F32, tag="g")
            emit_x(gA, gB, -1, True)

            # bias broadcast (K=1 matmul trick) into biasrep
            for ch in range(4):
                bps = pt.tile([128, 512], F32, tag="pt")
                nc.tensor.matmul(bps[:], onesv[:], browf[:, 512 * ch:512 * ch + 512],
                                 start=True, stop=True)
                nc.vector.tensor_copy(biasrep[:, 512 * ch:512 * ch + 512], bps[:])

            # flags: PSUM-needing pieces from the pt pool
            r0p = pt.tile([128, 1], F32, tag="pt")
            nc.tensor.matmul(r0p[:], Ss[:], isb[:, NS - 1:NS], start=True, stop=True)
            nc.vector.tensor_copy(r0col[:], r0p[:])
            nc.vector.tensor_copy(r0col[0:1, 0:1], d0f[:])
            nc.vector.tensor_scalar(notr[:, 0:1], r0col[:], -1.0, -1.0,
                                    op0=OP.mult, op1=OP.subtract)
            nc.vector.tensor_scalar(notr[:, 1:NS], isb[:, 0:NS - 1], -1.0, -1.0,
                                    op0=OP.mult, op1=OP.subtract)
            nc.vector.tensor_tensor(emit[:], isb[:], notr[:], op=OP.mult)
            nc.vector.tensor_tensor_scan(cums[:], emit[:], zero32[:], 0.0,
                                         op0=OP.add, op1=OP.add)
            offp = pt.tile([128, 1], F32, tag="pt")
            nc.tensor.matmul(offp[:], Ls[:], cums[:, NS - 1:NS], start=True, stop=True)
            nc.vector.tensor_copy(offs[:], offp[:])
            nc.vector.tensor_scalar(dest_f[:], cums[:], offs[:], -1.0,
                                    op0=OP.add, op1=OP.add)
            nc.vector.tensor_copy(emit_i[:], emit[:])
            nc.vector.copy_predicated(big[:], emit_i[:], dest_f[:])
            nc.vector.tensor_copy(dest_i[:], big[:])
            cntp = pt.tile([1, 1], F32, tag="pt")
            nc.tensor.matmul(cntp[:], ones128[:, 0:1], cums[:, NS - 1:NS],
                             start=True, stop=True)
            nc.vector.tensor_copy(cnt[:], cntp[:])

            for s in range(-1, NS):
                nr = zerocol if s == -1 else notr[:, s:s + 1]
                if s >= 0:
                    for half, gt in ((0, gA), (1, gB)):
                        for nch in range(2):
                            gsl = gt[:, 512 * nch:512 * nch + 512]
                            gw = slice(1024 * half + 512 * nch,
                                       1024 * half + 512 * nch + 512)
                            for kt in range(4):
                                nc.tensor.matmul(gsl, HT_prev[kt][:], whh_r[kt][:, gw],
                                                 start=False, stop=(kt == 3))
                # prefetch next step's X while this step's cell runs
                if s < NS - 1:
                    gA2 = pg.tile([128, 1024], F32, tag="g")
                    gB2 = pg.tile([128, 1024], F32, tag="g")
                    emit_x(gA2, gB2, s + 1, False)
                # cell: bias-add (DVE) then in-place activations (ACT)
                gi = acts.tile([128, 512], F32, tag="act")
                gf = acts.tile([128, 512], F32, tag="act")
                gg = acts.tile([128, 512], F32, tag="act")
                go_ = acts.tile([128, 512], F32, tag="act")
                nc.vector.tensor_add(gg[:], gB[:, 0:512], biasrep[:, 1024:1536])
                nc.vector.tensor_add(gi[:], gA[:, 0:512], biasrep[:, 0:512])
                nc.vector.tensor_add(gf[:], gA[:, 512:1024], biasrep[:, 512:1024])
                nc.vector.tensor_add(go_[:], gB[:, 512:1024], biasrep[:, 1536:2048])
                nc.scalar.activation(gg[:], gg[:], AF.Tanh)
                nc.scalar.activation(gi[:], gi[:], AF.Sigmoid)
                nc.scalar.activation(gf[:], gf[:], AF.Sigmoid)
                nc.scalar.activation(go_[:], go_[:], AF.Sigmoid)
                sig_i, sig_f, tgt, sig_o = gi, gf, gg, go_
                c_new = cpool.tile([128, 512], F32, tag="c")
                nc.vector.scalar_tensor_tensor(c_new[:], c_prev[:], nr[:], sig_f[:],
                                               op0=OP.mult, op1=OP.mult)
                pmul = acts.tile([128, 512], F32, tag="act")
                nc.vector.tensor_mul(pmul[:], sig_i[:], tgt[:])
                nc.vector.tensor_add(c_new[:], c_new[:], pmul[:])
                tc_t = acts.tile([128, 512], F32, tag="act")
                nc.scalar.activation(tc_t[:], c_new[:], AF.Tanh)

                if s >= 0:
                    h_t = hpool.tile([128, 512], F32, tag="h")
                    nc.gpsimd.tensor_mul(h_t[:], sig_o[:], tc_t[:])
                    nc.gpsimd.indirect_dma_start(
                        out=padded[:],
                        out_offset=IndirectOffsetOnAxis(ap=dest_i[:, s:s + 1], axis=0),
                        in_=h_t[:], in_offset=None,
                        bounds_check=breg, oob_is_err=False,
                    )
                    tvb = tvpool.tile([128, 32], F32, tag="tvb")
                    nc.scalar.activation(tvb[:], ones128[:, 0:32], AF.Copy,
                                         scale=tv[:, s:s + 1])
                    nc.gpsimd.indirect_dma_start(
                        out=B1[:],
                        out_offset=IndirectOffsetOnAxis(ap=dest_i[:, s:s + 1], axis=0),
                        in_=tvb[:], in_offset=None,
                        bounds_check=breg, oob_is_err=False,
                    )

                if s < NS - 1:
                    # next-step stationary: transpose of (sig_o * notr[s+1] * tanh_c)
                    h_m = hpool.tile([128, 512], F32, tag="hm")
                    for i in range(4):
                        csl = slice(128 * i, 128 * i + 128)
                        nc.vector.scalar_tensor_tensor(h_m[:, csl], sig_o[:, csl],
                                                       notr[:, s + 1:s + 2],
                                                       tc_t[:, csl],
                                                       op0=OP.mult, op1=OP.mult)
                    ptile = pt.tile([128, 512], F32, tag="pt")
                    for i in range(4):
                        nc.tensor.matmul(ptile[:, 128 * i:128 * i + 128],
                                         h_m[:, 128 * i:128 * i + 128], Id[:],
                                         is_transpose=True,
                                         start=(i == 0), stop=(i == 3))
                    HT = []
                    for i in range(4):
                        htt = htpool.tile([128, 128], F32R, tag="ht")
                        if i % 2 == 0:
                            nc.vector.tensor_copy(htt[:], ptile[:, 128 * i:128 * i + 128])
                        else:
                            nc.scalar.copy(htt[:], ptile[:, 128 * i:128 * i + 128])
                        HT.append(htt)
                    HT_prev = HT
                c_prev = c_new
                if s < NS - 1:
                    gA, gB = gA2, gB2

        # ================= Phase 4: bidx extraction =================
        with tc.tile_pool(name="bex", bufs=1) as bex, \
             tc.tile_pool(name="pbx", bufs=1, space="PSUM") as pbx:
            ext = bex.tile([128, 1024], F32, tag="ext")
            for bb in range(32):
                eng = nc.sync if bb % 2 == 0 else nc.scalar
                eng.dma_start(ext[:, 32 * bb:32 * bb + 32], B1r3[bb])
            bf = bex.tile([128, 32], F32, tag="bf")
            nc.vector.tensor_copy(bf[:], ext[:, 0:1024:32])
            bp = pbx.tile([32, 128], F32, tag="bp")
            nc.tensor.matmul(bp[:], bf[:], Id[:], is_transpose=True,
                             start=True, stop=True)
            bout = bex.tile([32, 128], F32, tag="bout")
            nc.vector.tensor_copy(bout[:], bp[:])
            fb = bex.tile([1, 1], F32, tag="fb")
            nc.vector.tensor_scalar(fb[:], cnt[:], 0.0, None, op0=OP.is_equal)
            fbi = bex.tile([1, 1], I32, tag="fbi")
            nc.vector.tensor_copy(fbi[:], fb[:])
            fbv = bex.tile([1, 1], F32, tag="fbv")
            nc.vector.memset(fbv[:], float(T - 1))
            nc.vector.copy_predicated(bout[0:1, 0:1], fbi[:], fbv[:])
            bi = bex.tile([32, 128], I32, tag="bi")
            nc.vector.tensor_copy(bi[:], bout[:])
            nc.sync.dma_start(bidx_o[:], bi[:])

    nc.compile()
    return nc


def _w23_65(W2):
    w = np.zeros((H1, 65), np.float32)
    w[:, 0] = W2[0]
    w[:, 32] = W2[1]
    w[:, 64] = W2[1] - W2[0]
    return w


def _bias65(b2):
    b = np.zeros((1, 65), np.float32)
    b[0, 0] = b2[0]
    b[0, 32] = b2[1]
    b[0, 64] = b2[1] - b2[0]
    return b


def make_host_inputs(x, W1, b1, W2, b2, W_ih, W_hh, b_ih, b_hh):
    """Returns per-core list of input dicts (host-side layout prep only)."""
    lanes = np.arange(128)
    lstrict = (lanes[:, None] < lanes[None, :]).astype(np.float32)      # L[q,p]=1 if q<p
    sshift = (lanes[:, None] == lanes[None, :] - 1).astype(np.float32)  # S[q,p]=1 if q==p-1
    ident = np.eye(128, dtype=np.float32)
    forced = np.zeros((128, NS), np.float32)
    forced[:, 31] = 1.0
    forced[127, 31] = 0.0
    forced[127, 30] = 1.0
    tvals = (32 * lanes[:, None] + 1 + np.arange(NS)[None, :]).astype(np.float32)
    common = {
        "wihT": np.ascontiguousarray(W_ih.T),
        "whhT": np.ascontiguousarray(W_hh.T),
        "w1T": np.ascontiguousarray(W1.T),
        "w23T": _w23_65(W2),
        "b1p": b1.reshape(2, 128, 1).astype(np.float32),
        "bias3": _bias65(b2),
        "brow": (b_ih + b_hh).reshape(1, G4).astype(np.float32),
        "lstrict": lstrict, "sshift": sshift, "ident": ident,
        "forced": forced, "tvals": tvals,
    }
    per_core = []
    for b in range(B):
        m = dict(common)
        m["xT"] = np.ascontiguousarray(x[b].T)
        per_core.append(m)
    return per_core


_NC_CACHE = {}


def kernel(x, W1, b1, W2, b2, W_ih, W_hh, b_ih, b_hh):
    from concourse.bass_utils import run_bass_kernel_spmd

    x = np.asarray(x, np.float32)
    args = [np.asarray(a, np.float32) for a in (W1, b1, W2, b2, W_ih, W_hh, b_ih, b_hh)]
    if "nc" not in _NC_CACHE:
        _NC_CACHE["nc"] = build_nc()
    nc = _NC_CACHE["nc"]
    in_maps = make_host_inputs(x, *args)
    res = run_bass_kernel_spmd(nc, in_maps, core_ids=list(range(B)))
    padded = np.stack([res.results[c]["padded"] for c in range(B)])
    bidx = np.stack([res.results[c]["bidx_o"].reshape(T) for c in range(B)])
    logits = np.stack([np.ascontiguousarray(res.results[c]["logits"].T)
                       for c in range(B)])
    return padded, bidx, logits


# revision 35
# speedup vs baseline: 1.0348x; 1.0164x over previous
"""AdaptiveSegmenter Trainium2 kernel (8 NeuronCores, pure data parallel).

Per core: one batch row.  Pipeline (all on device):
  1. Boundary MLP in exact fp32 (decision margins go down to ~4e-7, so the
     boundary path cannot tolerate float32r's ~1e-4 rounding):
     h1 = gelu(x@W1.T+b1); l01d = [l0, l1, l1-l0] = h1@[W2;w2d].T + b.
  2. Flag pipeline in "KS layout" [128, 32] (t = 32p+f+1): boundary flags,
     reset/emit, cumsum (tensor_tensor_scan + strict-lower-tri matmul),
     scatter destinations.
  3. Segmented LSTM as a 33-step chunk-parallel scan: lanes k=0..127 process
     t = 32k+1+s (s in [0,32)); extra step -1 covers t = 32k.  Forced
     boundaries every 32 steps make lanes independent; data-dependent resets
     are applied via per-lane masks.  Gates G.T [lanes, 4D] accumulate in
     PSUM: bias (K=1 ones trick) + X-part + H-part, float32r (full PE rate
     at N=512).  The hidden state is re-transposed each step by a fused
     mask-diag matmul (h.T @ diag(1-r)).
  4. Emitted hidden states + boundary time indices scattered to DRAM by
     row-granular indirect DMA with OOB-skip.  bidx extracted from a
     [4096, 128] scatter staging buffer.
Host side: only layout prep (transposes/stacking) and unshard.
"""
import os
import numpy as np
from contextlib import ExitStack

import concourse.bass as bass
import concourse.bacc as bacc
import concourse.tile as tile
from concourse import mybir
from concourse.bass import IndirectOffsetOnAxis

B, T, D = 8, 4096, 512
H1 = 256          # D // 2
G4 = 2048         # 4 * D
NS = 32           # steps per lane (s = 0..31); step -1 extra
F32 = mybir.dt.float32
F32R = mybir.dt.float32r
I32 = mybir.dt.int32
AF = mybir.ActivationFunctionType
OP = mybir.AluOpType

EXPLICIT_ZERO_FILL = bool(int(os.environ.get("KERNEL_ZERO_FILL", "1")))


def build_nc():
    nc = bacc.Bacc()
    # ---- parameters (per-core shard = one batch row; weights replicated) ----
    xT = nc.declare_dram_parameter("xT", [D, T], F32, isOutput=False)
    wihT = nc.declare_dram_parameter("wihT", [D, G4], F32, isOutput=False)
    whhT = nc.declare_dram_parameter("whhT", [D, G4], F32, isOutput=False)
    w1T = nc.declare_dram_parameter("w1T", [D, H1], F32, isOutput=False)
    w23T = nc.declare_dram_parameter("w23T", [H1, 65], F32, isOutput=False)
    b1p = nc.declare_dram_parameter("b1p", [2, 128, 1], F32, isOutput=False)
    bias3 = nc.declare_dram_parameter("bias3", [1, 65], F32, isOutput=False)
    brow = nc.declare_dram_parameter("brow", [1, G4], F32, isOutput=False)
    lstrict = nc.declare_dram_parameter("lstrict", [128, 128], F32, isOutput=False)
    sshift = nc.declare_dram_parameter("sshift", [128, 128], F32, isOutput=False)
    ident = nc.declare_dram_parameter("ident", [128, 128], F32, isOutput=False)
    forced = nc.declare_dram_parameter("forced", [128, NS], F32, isOutput=False)
    tvals = nc.declare_dram_parameter("tvals", [128, NS], F32, isOutput=False)

    padded = nc.declare_dram_parameter("padded", [T, D], F32, isOutput=True)
    bidx_o = nc.declare_dram_parameter("bidx_o", [32, 128], I32, isOutput=True)
    logits = nc.declare_dram_parameter("logits", [2, T], F32, isOutput=True)

    B1 = nc.dram_tensor("B1", [T, 32], F32)    # bidx scatter staging

    with tile.TileContext(nc) as tc, ExitStack() as top:
        sb = top.enter_context(tc.tile_pool(name="sb", bufs=1))

        # ---------- small constants ----------
        b1t = []
        for hh in range(2):
            bt = sb.tile([128, 1], F32, tag=f"b1_{hh}")
            nc.sync.dma_start(bt[:], b1p[hh])
            b1t.append(bt)
        Ls = sb.tile([128, 128], F32, tag="Ls")
        Ss = sb.tile([128, 128], F32, tag="Ss")
        Id = sb.tile([128, 128], F32, tag="Id")
        fo = sb.tile([128, NS], F32, tag="fo")
        tv = sb.tile([128, NS], F32, tag="tv")
        nc.sync.dma_start(Ls[:], lstrict[:])
        nc.sync.dma_start(Ss[:], sshift[:])
        nc.sync.dma_start(Id[:], ident[:])
        nc.sync.dma_start(fo[:], forced[:])
        nc.sync.dma_start(tv[:], tvals[:])
        ones128 = sb.tile([128, 128], F32, tag="ones128")
        nc.vector.memset(ones128[:], 1.0)
        neg1 = sb.tile([128, 128], F32, tag="neg1")
        nc.vector.memset(neg1[:], -1.0)
        zero_t = sb.tile([128, 512], F32, tag="zero_t")
        nc.vector.memset(zero_t[:], 0.0)

        # ---------- flag tiles (persistent; filled in phase 2) ----------
        dk = sb.tile([128, NS], F32, tag="dk")
        isb = sb.tile([128, NS], F32, tag="isb")
        notr = sb.tile([128, NS], F32, tag="notr")
        emit = sb.tile([128, NS], F32, tag="emit")
        cums = sb.tile([128, NS], F32, tag="cums")
        dest_f = sb.tile([128, NS], F32, tag="dest_f")
        dest_i = sb.tile([128, NS], I32, tag="dest_i")
        emit_i = sb.tile([128, NS], I32, tag="emit_i")
        big = sb.tile([128, NS], F32, tag="bigt")
        zero32 = sb.tile([128, NS], F32, tag="zero32")
        r0col = sb.tile([128, 1], F32, tag="r0col")
        offs = sb.tile([128, 1], F32, tag="offs")
        cnt = sb.tile([1, 1], F32, tag="cnt")
        zerocol = sb.tile([128, 1], F32, tag="zerocol")
        nc.vector.memset(zerocol[:], 0.0)
        nc.vector.memset(zero32[:], 0.0)
        nc.vector.memset(big[:], float(T))

        B1r3 = B1[:].rearrange("(a p) f -> a p f", p=128)
        scanp = top.enter_context(tc.tile_pool(name="scanp", bufs=1))

        # ================= Phase 0/1: x load (fp32) + MLP (fp32) ============
        xf_scope = ExitStack()
        mlp_scope = ExitStack()
        xfpool = xf_scope.enter_context(tc.tile_pool(name="xfpool", bufs=1))
        lpool = mlp_scope.enter_context(tc.tile_pool(name="lpool", bufs=1))
        x3 = xT[:].rearrange("(kt p) t -> kt p t", p=128)
        xf = []
        for kt in range(4):
            xft = xfpool.tile([128, T], F32, tag=f"xf{kt}")
            nc.sync.dma_start(xft[:], x3[kt])
            xf.append(xft)
        l01d = lpool.tile([65, T + 16], F32, tag="l01d")
        nc.vector.memset(l01d[64:65, T:T + 16], -1.0)


        with tc.tile_pool(name="mlpw", bufs=1) as mlpw, \
             tc.tile_pool(name="pmlp", bufs=2, space="PSUM") as pmlp, \
             tc.tile_pool(name="pmlp2", bufs=2, space="PSUM") as pmlp2, \
             tc.tile_pool(name="h1pool", bufs=4) as h1pool:
            w1r = []
            w13 = w1T[:].rearrange("(kt p) h -> kt p h", p=128)
            for kt in range(4):
                w1t_ = mlpw.tile([128, H1], F32, tag=f"w1{kt}")
                nc.sync.dma_start(w1t_[:], w13[kt])
                w1r.append(w1t_)
            w23r = []
            w233 = w23T[:].rearrange("(ht p) g -> ht p g", p=128)
            for ht in range(2):
                wt = mlpw.tile([128, 65], F32, tag=f"w23{ht}")
                nc.sync.dma_start(wt[:], w233[ht])
                w23r.append(wt)
            bias3f = mlpw.tile([1, 65], F32, tag="bias3f")
            nc.sync.dma_start(bias3f[:], bias3[:])
            ones_r = mlpw.tile([1, 512], F32, tag="ones_r")
            nc.vector.memset(ones_r[:], 1.0)

            # B1 prefill with -1 (bidx default); optional padded zero-fill
            for i in range(32):
                nc.sync.dma_start(B1r3[i], neg1[:, 0:32])
            if EXPLICIT_ZERO_FILL:
                pad3 = padded[:].rearrange("(a p) d -> a p d", p=128)
                for i in range(32):
                    nc.sync.dma_start(pad3[i], zero_t[:])

            # x -> f32r for the scan via SWDGE cast DMA (after MLP's HWDGE loads)
            xr = []
            for kt in range(4):
                xrt = scanp.tile([128, T + 32], F32R, tag=f"xr{kt}")
                nc.gpsimd.dma_start(xrt[:, 0:T], x3[kt])
                nc.vector.tensor_copy(xrt[:, T:T + 32], zero_t[:, 0:32])
                xr.append(xrt)

            for tci in range(8):
                tsl = slice(512 * tci, 512 * tci + 512)
                h1c = []
                for hh in range(2):
                    ps1 = pmlp.tile([128, 512], F32, tag="ps1")
                    for kt in range(4):
                        nc.tensor.matmul(ps1[:], w1r[kt][:, 128 * hh:128 * hh + 128],
                                         xf[kt][:, tsl], start=(kt == 0), stop=(kt == 3))
                    hc = h1pool.tile([128, 512], F32, tag="h1c")
                    nc.scalar.activation(hc[:], ps1[:], AF.Gelu, bias=b1t[hh][:])
                    h1c.append(hc)
                ps2 = pmlp2.tile([65, 512], F32, tag="ps2")
                nc.tensor.matmul(ps2[:], bias3f[:], ones_r[:], start=True, stop=False)
                for hh in range(2):
                    nc.tensor.matmul(ps2[:], w23r[hh][:], h1c[hh][:],
                                     start=False, stop=(hh == 1))
                nc.vector.tensor_copy(l01d[:, tsl], ps2[:])
        nc.sync.dma_start(logits[:], l01d[0:64:32, 0:T])

        # ================= Phase 2: flags — DVE prep (no PSUM) =============
        nc.sync.dma_start(dk[:], l01d[64:65, 1:T + 1])
        nc.vector.tensor_scalar(dk[:], dk[:], 0.0, None, op0=OP.is_gt)
        nc.vector.tensor_tensor(isb[:], dk[:], fo[:], op=OP.max)
        d0f = sb.tile([1, 1], F32, tag="d0f")
        nc.vector.tensor_scalar(d0f[:], l01d[64:65, 0:1], 0.0, None, op0=OP.is_gt)
        mlp_scope.close()
        xf_scope.close()

        # ================= Phase 2b: W loads (f32r, SWDGE cast) =============
        wpool = top.enter_context(tc.tile_pool(name="wpool", bufs=1))
        biasrep = wpool.tile([128, G4], F32, tag="biasrep")
        onesv = wpool.tile([1, 128], F32, tag="onesv")
        browf = wpool.tile([1, G4], F32, tag="browf")
        nc.sync.dma_start(browf[:], brow[:])
        nc.vector.tensor_copy(onesv[:], ones128[0:1, 0:128])
        wih_r, whh_r = [], []
        for src, dst_list, nm in ((wihT, wih_r, "wih"), (whhT, whh_r, "whh")):
            w3 = src[:].rearrange("(kt p) g -> kt p g", p=128)
            for kt in range(4):
                wr = wpool.tile([128, G4], F32R, tag=f"{nm}{kt}")
                nc.gpsimd.dma_start(wr[:], w3[kt])
                dst_list.append(wr)

        # ================= Phase 3: scan =================
        with tc.tile_pool(name="pg", bufs=3, space="PSUM") as pg, \
             tc.tile_pool(name="pt", bufs=2, space="PSUM") as pt, \
             tc.tile_pool(name="acts", bufs=12) as acts, \
             tc.tile_pool(name="cpool", bufs=3) as cpool, \
             tc.tile_pool(name="hpool", bufs=4) as hpool, \
             tc.tile_pool(name="htpool", bufs=8) as htpool, \
             tc.tile_pool(name="tvpool", bufs=3) as tvpool:

            c_prev = cpool.tile([128, 512], F32, tag="c")
            nc.vector.memset(c_prev[:], 0.0)
            HT_prev = None
            breg = nc.gpsimd.to_reg(T - 1)

            def emit_x(gAt, gBt, st, stop):
                xsl = slice(1 + st, 1 + st + 32 * 128, 32)
                for half, gt in ((0, gAt), (1, gBt)):
                    for nch in range(2):
                        gsl = gt[:, 512 * nch:512 * nch + 512]
                        gw = slice(1024 * half + 512 * nch,
                                   1024 * half + 512 * nch + 512)
                        for kt in range(4):
                            nc.tensor.matmul(gsl, xr[kt][:, xsl], wih_r[kt][:, gw],
                                             start=(kt == 0), stop=(stop and kt == 3))

            # prefetch X for step -1 (PE busy while flags compute on DVE)
            gA = pg.tile([128, 1024], F32, tag="g")
            gB = pg.tile([128, 1024], F32, tag="g")
            emit_x(gA, gB, -1, True)

            # bias broadcast (K=1 matmul trick) into biasrep
            for ch in range(4):
                bps = pt.tile([128, 512], F32, tag="pt")
                nc.tensor.matmul(bps[:], onesv[:], browf[:, 512 * ch:512 * ch + 512],
                                 start=True, stop=True)
                nc.vector.tensor_copy(biasrep[:, 512 * ch:512 * ch + 512], bps[:])

            # flags: PSUM-needing pieces from the pt pool
            r0p = pt.tile([128, 1], F32, tag="pt")
            nc.tensor.matmul(r0p[:], Ss[:], isb[:, NS - 1:NS], start=True, stop=True)
            nc.vector.tensor_copy(r0col[:], r0p[:])
            nc.vector.tensor_copy(r0col[0:1, 0:1], d0f[:])
            nc.vector.tensor_scalar(notr[:, 0:1], r0col[:], -1.0, -1.0,
                                    op0=OP.mult, op1=OP.subtract)
            nc.vector.tensor_scalar(notr[:, 1:NS], isb[:, 0:NS - 1], -1.0, -1.0,
                                    op0=OP.mult, op1=OP.subtract)
            nc.vector.tensor_tensor(emit[:], isb[:], notr[:], op=OP.mult)
            nc.vector.tensor_tensor_scan(cums[:], emit[:], zero32[:], 0.0,
                                         op0=OP.add, op1=OP.add)
            offp = pt.tile([128, 1], F32, tag="pt")
            nc.tensor.matmul(offp[:], Ls[:], cums[:, NS - 1:NS], start=True, stop=True)
            nc.vector.tensor_copy(offs[:], offp[:])
            nc.vector.tensor_scalar(dest_f[:], cums[:], offs[:], -1.0,
                                    op0=OP.add, op1=OP.add)
            nc.vector.tensor_copy(emit_i[:], emit[:])
            nc.vector.copy_predicated(big[:], emit_i[:], dest_f[:])
            nc.vector.tensor_copy(dest_i[:], big[:])
            cntp = pt.tile([1, 1], F32, tag="pt")
            nc.tensor.matmul(cntp[:], ones128[:, 0:1], cums[:, NS - 1:NS],
                             start=True, stop=True)
            nc.vector.tensor_copy(cnt[:], cntp[:])

            for s in range(-1, NS):
                nr = zerocol if s == -1 else notr[:, s:s + 1]
                if s >= 0:
                    for half, gt in ((0, gA), (1, gB)):
                        for nch in range(2):
                            gsl = gt[:, 512 * nch:512 * nch + 512]
                            gw = slice(1024 * half + 512 * nch,
                                       1024 * half + 512 * nch + 512)
                            for kt in range(4):
                                nc.tensor.matmul(gsl, HT_prev[kt][:], whh_r[kt][:, gw],
                                                 start=False, stop=(kt == 3))
                # prefetch next step's X while this step's cell runs
                if s < NS - 1:
                    gA2 = pg.tile([128, 1024], F32, tag="g")
                    gB2 = pg.tile([128, 1024], F32, tag="g")
                    emit_x(gA2, gB2, s + 1, False)
                # cell: bias-add (DVE) then in-place activations (ACT)
                gi = acts.tile([128, 512], F32, tag="act")
                gf = acts.tile([128, 512], F32, tag="act")
                gg = acts.tile([128, 512], F32, tag="act")
                go_ = acts.tile([128, 512], F32, tag="act")
                nc.vector.tensor_add(gg[:], gB[:, 0:512], biasrep[:, 1024:1536])
                nc.vector.tensor_add(gi[:], gA[:, 0:512], biasrep[:, 0:512])
                nc.vector.tensor_add(gf[:], gA[:, 512:1024], biasrep[:, 512:1024])
                nc.vector.tensor_add(go_[:], gB[:, 512:1024], biasrep[:, 1536:2048])
                nc.scalar.activation(gg[:], gg[:], AF.Tanh)
                nc.scalar.activation(gi[:], gi[:], AF.Sigmoid)
                nc.scalar.activation(gf[:], gf[:], AF.Sigmoid)
                nc.scalar.activation(go_[:], go_[:], AF.Sigmoid)
                sig_i, sig_f, tgt, sig_o = gi, gf, gg, go_
                c_new = cpool.tile([128, 512], F32, tag="c")
                nc.vector.scalar_tensor_tensor(c_new[:], c_prev[:], nr[:], sig_f[:],
                                               op0=OP.mult, op1=OP.mult)
                pmul = acts.tile([128, 512], F32, tag="act")
                nc.vector.tensor_mul(pmul[:], sig_i[:], tgt[:])
                nc.vector.tensor_add(c_new[:], c_new[:], pmul[:])
                tc_t = acts.tile([128, 512], F32, tag="act")
                nc.scalar.activation(tc_t[:], c_new[:], AF.Tanh)

                if s >= 0:
                    h_t = hpool.tile([128, 512], F32, tag="h")
                    nc.gpsimd.tensor_mul(h_t[:], sig_o[:], tc_t[:])
                    nc.gpsimd.indirect_dma_start(
                        out=padded[:],
                        out_offset=IndirectOffsetOnAxis(ap=dest_i[:, s:s + 1], axis=0),
                        in_=h_t[:], in_offset=None,
                        bounds_check=breg, oob_is_err=False,
                    )
                    tvb = tvpool.tile([128, 32], F32, tag="tvb")
                    nc.scalar.activation(tvb[:], ones128[:, 0:32], AF.Copy,
                                         scale=tv[:, s:s + 1])
                    nc.gpsimd.indirect_dma_start(
                        out=B1[:],
                        out_offset=IndirectOffsetOnAxis(ap=dest_i[:, s:s + 1], axis=0),
                        in_=tvb[:], in_offset=None,
                        bounds_check=breg, oob_is_err=False,
                    )

                if s < NS - 1:
                    # next-step stationary: transpose of (sig_o * notr[s+1] * tanh_c)
                    h_m = hpool.tile([128, 512], F32, tag="hm")
                    for i in range(4):
                        csl = slice(128 * i, 128 * i + 128)
                        nc.vector.scalar_tensor_tensor(h_m[:, csl], sig_o[:, csl],
                                                       notr[:, s + 1:s + 2],
                                                       tc_t[:, csl],
                                                       op0=OP.mult, op1=OP.mult)
                    ptile = pt.tile([128, 512], F32, tag="pt")
                    for i in range(4):
                        nc.tensor.matmul(ptile[:, 128 * i:128 * i + 128],
                                         h_m[:, 128 * i:128 * i + 128], Id[:],
                                         is_transpose=True,
                                         start=(i == 0), stop=(i == 3))
                    HT = []
                    for i in range(4):
                        htt = htpool.tile([128, 128], F32R, tag="ht")
                        if i % 2 == 0:
                            nc.vector.tensor_copy(htt[:], ptile[:, 128 * i:128 * i + 128])
                        else:
                            nc.scalar.copy(htt[:], ptile[:, 128 * i:128 * i + 128])
                        HT.append(htt)
                    HT_prev = HT
                c_prev = c_new
                if s < NS - 1:
                    gA, gB = gA2, gB2

        # ================= Phase 4: bidx extraction =================
        with tc.tile_pool(name="bex", bufs=1) as bex, \
             tc.tile_pool(name="pbx", bufs=1, space="PSUM") as pbx:
            ext = bex.tile([128, 1024], F32, tag="ext")
            for bb in range(32):
                eng = nc.sync if bb % 2 == 0 else nc.scalar
                eng.dma_start(ext[:, 32 * bb:32 * bb + 32], B1r3[bb])
            bf = bex.tile([128, 32], F32, tag="bf")
            nc.vector.tensor_copy(bf[:], ext[:, 0:1024:32])
            bp = pbx.tile([32, 128], F32, tag="bp")
            nc.tensor.matmul(bp[:], bf[:], Id[:], is_transpose=True,
                             start=True, stop=True)
            bout = bex.tile([32, 128], F32, tag="bout")
            nc.vector.tensor_copy(bout[:], bp[:])
            fb = bex.tile([1, 1], F32, tag="fb")
            nc.vector.tensor_scalar(fb[:], cnt[:], 0.0, None, op0=OP.is_equal)
            fbi = bex.tile([1, 1], I32, tag="fbi")
            nc.vector.tensor_copy(fbi[:], fb[:])
            fbv = bex.tile([1, 1], F32, tag="fbv")
            nc.vector.memset(fbv[:], float(T - 1))
            nc.vector.copy_predicated(bout[0:1, 0:1], fbi[:], fbv[:])
            bi = bex.tile([32, 128], I32, tag="bi")
            nc.vector.tensor_copy(bi[:], bout[:])
            nc.sync.dma_start(bidx_o[:], bi[:])

    nc.compile()
    return nc


def _w23_65(W2):
    w = np.zeros((H1, 65), np.float32)
    w[:, 0] = W2[0]
    w[:, 32] = W2[1]
    w[:, 64] = W2[1] - W2[0]
    return w


def _bias65(b2):
    b = np.zeros((1, 65), np.float32)
    b[0, 0] = b2[0]
    b[0, 32] = b2[1]
    b[0, 64] = b2[1] - b2[0]
    return b


def make_host_inputs(x, W1, b1, W2, b2, W_ih, W_hh, b_ih, b_hh):
    """Returns per-core list of input dicts (host-side layout prep only)."""
    lanes = np.arange(128)
    lstrict = (lanes[:, None] < lanes[None, :]).astype(np.float32)      # L[q,p]=1 if q<p
    sshift = (lanes[:, None] == lanes[None, :] - 1).astype(np.float32)  # S[q,p]=1 if q==p-1
    ident = np.eye(128, dtype=np.float32)
    forced = np.zeros((128, NS), np.float32)
    forced[:, 31] = 1.0
    forced[127, 31] = 0.0
    forced[127, 30] = 1.0
    tvals = (32 * lanes[:, None] + 1 + np.arange(NS)[None, :]).astype(np.float32)
    common = {
        "wihT": np.ascontiguousarray(W_ih.T),
        "whhT": np.ascontiguousarray(W_hh.T),
        "w1T": np.ascontiguousarray(W1.T),
        "w23T": _w23_65(W2),
        "b1p": b1.reshape(2, 128, 1).astype(np.float32),
        "bias3": _bias65(b2),
        "brow": (b_ih + b_hh).reshape(1, G4).astype(np.float32),
        "lstrict": lstrict, "sshift": sshift, "ident": ident,
        "forced": forced, "tvals": tvals,
    }
    per_core = []
    for b in range(B):
        m = dict(common)
        m["xT"] = np.ascontiguousarray(x[b].T)
        per_core.append(m)
    return per_core


_NC_CACHE = {}


def kernel(x, W1, b1, W2, b2, W_ih, W_hh, b_ih, b_hh):
    from concourse.bass_utils import run_bass_kernel_spmd

    x = np.asarray(x, np.float32)
    args = [np.asarray(a, np.float32) for a in (W1, b1, W2, b2, W_ih, W_hh, b_ih, b_hh)]
    if "nc" not in _NC_CACHE:
        _NC_CACHE["nc"] = build_nc()
    nc = _NC_CACHE["nc"]
    in_maps = make_host_inputs(x, *args)
    res = run_bass_kernel_spmd(nc, in_maps, core_ids=list(range(B)))
    padded = np.stack([res.results[c]["padded"] for c in range(B)])
    bidx = np.stack([res.results[c]["bidx_o"].reshape(T) for c in range(B)])
    logits = np.stack([np.ascontiguousarray(res.results[c]["logits"].T)
                       for c in range(B)])
    return padded, bidx, logits


# revision 36
# speedup vs baseline: 1.0899x; 1.0533x over previous
"""AdaptiveSegmenter Trainium2 kernel (8 NeuronCores, pure data parallel).

Per core: one batch row.  Pipeline (all on device):
  1. Boundary MLP in exact fp32 (decision margins go down to ~4e-7, so the
     boundary path cannot tolerate float32r's ~1e-4 rounding):
     h1 = gelu(x@W1.T+b1); l01d = [l0, l1, l1-l0] = h1@[W2;w2d].T + b.
  2. Flag pipeline in "KS layout" [128, 32] (t = 32p+f+1): boundary flags,
     reset/emit, cumsum (tensor_tensor_scan + strict-lower-tri matmul),
     scatter destinations.
  3. Segmented LSTM as a 33-step chunk-parallel scan: lanes k=0..127 process
     t = 32k+1+s (s in [0,32)); extra step -1 covers t = 32k.  Forced
     boundaries every 32 steps make lanes independent; data-dependent resets
     are applied via per-lane masks.  Gates G.T [lanes, 4D] accumulate in
     PSUM: bias (K=1 ones trick) + X-part + H-part, float32r (full PE rate
     at N=512).  The hidden state is re-transposed each step by a fused
     mask-diag matmul (h.T @ diag(1-r)).
  4. Emitted hidden states + boundary time indices scattered to DRAM by
     row-granular indirect DMA with OOB-skip.  bidx extracted from a
     [4096, 128] scatter staging buffer.
Host side: only layout prep (transposes/stacking) and unshard.
"""
import os
import numpy as np
from contextlib import ExitStack

import concourse.bass as bass
import concourse.bacc as bacc
import concourse.tile as tile
from concourse import mybir
from concourse.bass import IndirectOffsetOnAxis

B, T, D = 8, 4096, 512
H1 = 256          # D // 2
G4 = 2048         # 4 * D
NS = 32           # steps per lane (s = 0..31); step -1 extra
F32 = mybir.dt.float32
F32R = mybir.dt.float32r
I32 = mybir.dt.int32
AF = mybir.ActivationFunctionType
OP = mybir.AluOpType

EXPLICIT_ZERO_FILL = bool(int(os.environ.get("KERNEL_ZERO_FILL", "1")))


def build_nc():
    nc = bacc.Bacc()
    # ---- parameters (per-core shard = one batch row; weights replicated) ----
    xT = nc.declare_dram_parameter("xT", [D, T], F32, isOutput=False)
    wihT = nc.declare_dram_parameter("wihT", [D, G4], F32, isOutput=False)
    whhT = nc.declare_dram_parameter("whhT", [D, G4], F32, isOutput=False)
    w1T = nc.declare_dram_parameter("w1T", [D, H1], F32, isOutput=False)
    w23T = nc.declare_dram_parameter("w23T", [H1, 65], F32, isOutput=False)
    b1p = nc.declare_dram_parameter("b1p", [2, 128, 1], F32, isOutput=False)
    bias3 = nc.declare_dram_parameter("bias3", [1, 65], F32, isOutput=False)
    brow = nc.declare_dram_parameter("brow", [1, G4], F32, isOutput=False)
    lstrict = nc.declare_dram_parameter("lstrict", [128, 128], F32, isOutput=False)
    sshift = nc.declare_dram_parameter("sshift", [128, 128], F32, isOutput=False)
    ident = nc.declare_dram_parameter("ident", [128, 128], F32, isOutput=False)
    forced = nc.declare_dram_parameter("forced", [128, NS], F32, isOutput=False)
    tvals = nc.declare_dram_parameter("tvals", [128, NS], F32, isOutput=False)

    padded = nc.declare_dram_parameter("padded", [T, D], F32, isOutput=True)
    bidx_o = nc.declare_dram_parameter("bidx_o", [32, 128], I32, isOutput=True)
    logits = nc.declare_dram_parameter("logits", [2, T], F32, isOutput=True)

    B1 = nc.dram_tensor("B1", [T, 32], F32)    # bidx scatter staging

    with tile.TileContext(nc) as tc, ExitStack() as top:
        sb = top.enter_context(tc.tile_pool(name="sb", bufs=1))

        # ---------- small constants ----------
        b1t = []
        for hh in range(2):
            bt = sb.tile([128, 1], F32, tag=f"b1_{hh}")
            nc.sync.dma_start(bt[:], b1p[hh])
            b1t.append(bt)
        Ls = sb.tile([128, 128], F32, tag="Ls")
        Ss = sb.tile([128, 128], F32, tag="Ss")
        Id = sb.tile([128, 128], F32, tag="Id")
        fo = sb.tile([128, NS], F32, tag="fo")
        tv = sb.tile([128, NS], F32, tag="tv")
        nc.sync.dma_start(Ls[:], lstrict[:])
        nc.sync.dma_start(Ss[:], sshift[:])
        nc.sync.dma_start(Id[:], ident[:])
        nc.sync.dma_start(fo[:], forced[:])
        nc.sync.dma_start(tv[:], tvals[:])
        ones128 = sb.tile([128, 128], F32, tag="ones128")
        nc.vector.memset(ones128[:], 1.0)
        neg1 = sb.tile([128, 128], F32, tag="neg1")
        nc.vector.memset(neg1[:], -1.0)
        zero_t = sb.tile([128, 512], F32, tag="zero_t")
        nc.vector.memset(zero_t[:], 0.0)

        # ---------- flag tiles (persistent; filled in phase 2) ----------
        dk = sb.tile([128, NS], F32, tag="dk")
        isb = sb.tile([128, NS], F32, tag="isb")
        notr = sb.tile([128, NS], F32, tag="notr")
        emit = sb.tile([128, NS], F32, tag="emit")
        cums = sb.tile([128, NS], F32, tag="cums")
        dest_f = sb.tile([128, NS], F32, tag="dest_f")
        dest_i = sb.tile([128, NS], I32, tag="dest_i")
        emit_i = sb.tile([128, NS], I32, tag="emit_i")
        big = sb.tile([128, NS], F32, tag="bigt")
        zero32 = sb.tile([128, NS], F32, tag="zero32")
        r0col = sb.tile([128, 1], F32, tag="r0col")
        offs = sb.tile([128, 1], F32, tag="offs")
        cnt = sb.tile([1, 1], F32, tag="cnt")
        zerocol = sb.tile([128, 1], F32, tag="zerocol")
        nc.vector.memset(zerocol[:], 0.0)
        nc.vector.memset(zero32[:], 0.0)
        nc.vector.memset(big[:], float(T))

        B1r3 = B1[:].rearrange("(a p) f -> a p f", p=128)
        scanp = top.enter_context(tc.tile_pool(name="scanp", bufs=1))

        # ================= Phase 0/1: x load (fp32) + MLP (fp32) ============
        xf_scope = ExitStack()
        mlp_scope = ExitStack()
        xfpool = xf_scope.enter_context(tc.tile_pool(name="xfpool", bufs=1))
        lpool = mlp_scope.enter_context(tc.tile_pool(name="lpool", bufs=1))
        x3 = xT[:].rearrange("(kt p) t -> kt p t", p=128)
        xf = []
        for kt in range(4):
            xft = xfpool.tile([128, T], F32, tag=f"xf{kt}")
            nc.sync.dma_start(xft[:], x3[kt])
            xf.append(xft)
        l01d = lpool.tile([65, T + 16], F32, tag="l01d")
        nc.vector.memset(l01d[64:65, T:T + 16], -1.0)


        with tc.tile_pool(name="mlpw", bufs=1) as mlpw, \
             tc.tile_pool(name="pmlp", bufs=2, space="PSUM") as pmlp, \
             tc.tile_pool(name="pmlp2", bufs=2, space="PSUM") as pmlp2, \
             tc.tile_pool(name="h1pool", bufs=4) as h1pool:
            w1r = []
            w13 = w1T[:].rearrange("(kt p) h -> kt p h", p=128)
            for kt in range(4):
                w1t_ = mlpw.tile([128, H1], F32, tag=f"w1{kt}")
                nc.sync.dma_start(w1t_[:], w13[kt])
                w1r.append(w1t_)
            w23r = []
            w233 = w23T[:].rearrange("(ht p) g -> ht p g", p=128)
            for ht in range(2):
                wt = mlpw.tile([128, 65], F32, tag=f"w23{ht}")
                nc.sync.dma_start(wt[:], w233[ht])
                w23r.append(wt)
            bias3f = mlpw.tile([1, 65], F32, tag="bias3f")
            nc.sync.dma_start(bias3f[:], bias3[:])
            ones_r = mlpw.tile([1, 512], F32, tag="ones_r")
            nc.vector.memset(ones_r[:], 1.0)

            # B1 prefill with -1 (bidx default); optional padded zero-fill
            for i in range(32):
                nc.sync.dma_start(B1r3[i], neg1[:, 0:32])
            if EXPLICIT_ZERO_FILL:
                pad3 = padded[:].rearrange("(a p) d -> a p d", p=128)
                for i in range(32):
                    nc.sync.dma_start(pad3[i], zero_t[:])

            # x -> f32r for the scan via SWDGE cast DMA (after MLP's HWDGE loads)
            xr = []
            for kt in range(4):
                xrt = scanp.tile([128, T + 32], F32R, tag=f"xr{kt}")
                nc.gpsimd.dma_start(xrt[:, 0:T], x3[kt])
                nc.vector.tensor_copy(xrt[:, T:T + 32], zero_t[:, 0:32])
                xr.append(xrt)

            for tci in range(8):
                tsl = slice(512 * tci, 512 * tci + 512)
                h1c = []
                for hh in range(2):
                    ps1 = pmlp.tile([128, 512], F32, tag="ps1")
                    for kt in range(4):
                        nc.tensor.matmul(ps1[:], w1r[kt][:, 128 * hh:128 * hh + 128],
                                         xf[kt][:, tsl], start=(kt == 0), stop=(kt == 3))
                    hc = h1pool.tile([128, 512], F32, tag="h1c")
                    nc.scalar.activation(hc[:], ps1[:], AF.Gelu, bias=b1t[hh][:])
                    h1c.append(hc)
                ps2 = pmlp2.tile([65, 512], F32, tag="ps2")
                nc.tensor.matmul(ps2[:], bias3f[:], ones_r[:], start=True, stop=False)
                for hh in range(2):
                    nc.tensor.matmul(ps2[:], w23r[hh][:], h1c[hh][:],
                                     start=False, stop=(hh == 1))
                nc.vector.tensor_copy(l01d[:, tsl], ps2[:])
        nc.sync.dma_start(logits[:], l01d[0:64:32, 0:T])

        # ================= Phase 2: flags — DVE prep (no PSUM) =============
        nc.sync.dma_start(dk[:], l01d[64:65, 1:T + 1])
        nc.vector.tensor_scalar(dk[:], dk[:], 0.0, None, op0=OP.is_gt)
        nc.vector.tensor_tensor(isb[:], dk[:], fo[:], op=OP.max)
        d0f = sb.tile([1, 1], F32, tag="d0f")
        nc.vector.tensor_scalar(d0f[:], l01d[64:65, 0:1], 0.0, None, op0=OP.is_gt)
        mlp_scope.close()
        xf_scope.close()

        # ================= Phase 2b: W loads (f32r, SWDGE cast) =============
        wpool = top.enter_context(tc.tile_pool(name="wpool", bufs=1))
        biasrep = wpool.tile([128, G4], F32, tag="biasrep")
        onesv = wpool.tile([1, 128], F32, tag="onesv")
        browf = wpool.tile([1, G4], F32, tag="browf")
        nc.sync.dma_start(browf[:], brow[:])
        nc.vector.tensor_copy(onesv[:], ones128[0:1, 0:128])
        wih_r, whh_r = [], []
        for src, dst_list, nm in ((wihT, wih_r, "wih"), (whhT, whh_r, "whh")):
            w3 = src[:].rearrange("(kt p) g -> kt p g", p=128)
            for kt in range(4):
                wr = wpool.tile([128, G4], F32R, tag=f"{nm}{kt}")
                nc.gpsimd.dma_start(wr[:], w3[kt])
                dst_list.append(wr)

        # ================= Phase 3: scan =================
        with tc.tile_pool(name="pg", bufs=3, space="PSUM") as pg, \
             tc.tile_pool(name="pt", bufs=2, space="PSUM") as pt, \
             tc.tile_pool(name="acts", bufs=12) as acts, \
             tc.tile_pool(name="cpool", bufs=3) as cpool, \
             tc.tile_pool(name="hpool", bufs=4) as hpool, \
             tc.tile_pool(name="htpool", bufs=8) as htpool, \
             tc.tile_pool(name="tvpool", bufs=3) as tvpool:

            c_prev = cpool.tile([128, 512], F32, tag="c")
            nc.vector.memset(c_prev[:], 0.0)
            HT_prev = None
            breg = nc.gpsimd.to_reg(T - 1)

            def emit_x(gAt, gBt, st, stop):
                xsl = slice(1 + st, 1 + st + 32 * 128, 32)
                for half, gt in ((0, gAt), (1, gBt)):
                    for nch in range(2):
                        gsl = gt[:, 512 * nch:512 * nch + 512]
                        gw = slice(1024 * half + 512 * nch,
                                   1024 * half + 512 * nch + 512)
                        for kt in range(4):
                            nc.tensor.matmul(gsl, xr[kt][:, xsl], wih_r[kt][:, gw],
                                             start=(kt == 0), stop=(stop and kt == 3))

            # prefetch X for step -1 (PE busy while flags compute on DVE)
            gA = pg.tile([128, 1024], F32, tag="g")
            gB = pg.tile([128, 1024], F32, tag="g")
            emit_x(gA, gB, -1, True)

            # bias broadcast (K=1 matmul trick) into biasrep
            for ch in range(4):
                bps = pt.tile([128, 512], F32, tag="pt")
                nc.tensor.matmul(bps[:], onesv[:], browf[:, 512 * ch:512 * ch + 512],
                                 start=True, stop=True)
                nc.vector.tensor_copy(biasrep[:, 512 * ch:512 * ch + 512], bps[:])

            # flags: PSUM-needing pieces from the pt pool
            r0p = pt.tile([128, 1], F32, tag="pt")
            nc.tensor.matmul(r0p[:], Ss[:], isb[:, NS - 1:NS], start=True, stop=True)
            nc.vector.tensor_copy(r0col[:], r0p[:])
            nc.vector.tensor_copy(r0col[0:1, 0:1], d0f[:])
            nc.vector.tensor_scalar(notr[:, 0:1], r0col[:], -1.0, -1.0,
                                    op0=OP.mult, op1=OP.subtract)
            nc.vector.tensor_scalar(notr[:, 1:NS], isb[:, 0:NS - 1], -1.0, -1.0,
                                    op0=OP.mult, op1=OP.subtract)
            nc.vector.tensor_tensor(emit[:], isb[:], notr[:], op=OP.mult)
            nc.vector.tensor_tensor_scan(cums[:], emit[:], zero32[:], 0.0,
                                         op0=OP.add, op1=OP.add)
            offp = pt.tile([128, 1], F32, tag="pt")
            nc.tensor.matmul(offp[:], Ls[:], cums[:, NS - 1:NS], start=True, stop=True)
            nc.vector.tensor_copy(offs[:], offp[:])
            nc.vector.tensor_scalar(dest_f[:], cums[:], offs[:], -1.0,
                                    op0=OP.add, op1=OP.add)
            nc.vector.tensor_copy(emit_i[:], emit[:])
            nc.vector.copy_predicated(big[:], emit_i[:], dest_f[:])
            nc.vector.tensor_copy(dest_i[:], big[:])
            cntp = pt.tile([1, 1], F32, tag="pt")
            nc.tensor.matmul(cntp[:], ones128[:, 0:1], cums[:, NS - 1:NS],
                             start=True, stop=True)
            nc.vector.tensor_copy(cnt[:], cntp[:])

            for s in range(-1, NS):
                nr = zerocol if s == -1 else notr[:, s:s + 1]
                if s >= 0:
                    for half, gt in ((0, gA), (1, gB)):
                        for nch in range(2):
                            gsl = gt[:, 512 * nch:512 * nch + 512]
                            gw = slice(1024 * half + 512 * nch,
                                       1024 * half + 512 * nch + 512)
                            for kt in range(4):
                                nc.tensor.matmul(gsl, HT_prev[kt][:], whh_r[kt][:, gw],
                                                 start=False, stop=(kt == 3))
                # prefetch next step's X while this step's cell runs
                if s < NS - 1:
                    gA2 = pg.tile([128, 1024], F32, tag="g")
                    gB2 = pg.tile([128, 1024], F32, tag="g")
                    emit_x(gA2, gB2, s + 1, False)
                # cell: bias-add (DVE) then in-place activations (ACT)
                gi = acts.tile([128, 512], F32, tag="act")
                gf = acts.tile([128, 512], F32, tag="act")
                gg = acts.tile([128, 512], F32, tag="act")
                go_ = acts.tile([128, 512], F32, tag="act")
                nc.vector.tensor_add(gg[:], gB[:, 0:512], biasrep[:, 1024:1536])
                nc.vector.tensor_add(gi[:], gA[:, 0:512], biasrep[:, 0:512])
                nc.vector.tensor_add(gf[:], gA[:, 512:1024], biasrep[:, 512:1024])
                nc.vector.tensor_add(go_[:], gB[:, 512:1024], biasrep[:, 1536:2048])
                nc.scalar.activation(gg[:], gg[:], AF.Tanh)
                nc.scalar.activation(gi[:], gi[:], AF.Sigmoid)
                nc.scalar.activation(gf[:], gf[:], AF.Sigmoid)
                nc.scalar.activation(go_[:], go_[:], AF.Sigmoid)
                sig_i, sig_f, tgt, sig_o = gi, gf, gg, go_
                c_new = cpool.tile([128, 512], F32, tag="c")
                nc.vector.scalar_tensor_tensor(c_new[:], c_prev[:], nr[:], sig_f[:],
                                               op0=OP.mult, op1=OP.mult)
                pmul = acts.tile([128, 512], F32, tag="act")
                nc.vector.tensor_mul(pmul[:], sig_i[:], tgt[:])
                nc.vector.tensor_add(c_new[:], c_new[:], pmul[:])
                tc_t = acts.tile([128, 512], F32, tag="act")
                for qq in range(4):
                    qsl = slice(128 * qq, 128 * qq + 128)
                    nc.scalar.activation(tc_t[:, qsl], c_new[:, qsl], AF.Tanh)

                if s >= 0:
                    h_t = hpool.tile([128, 512], F32, tag="h")
                    nc.gpsimd.tensor_mul(h_t[:], sig_o[:], tc_t[:])
                    nc.gpsimd.indirect_dma_start(
                        out=padded[:],
                        out_offset=IndirectOffsetOnAxis(ap=dest_i[:, s:s + 1], axis=0),
                        in_=h_t[:], in_offset=None,
                        bounds_check=breg, oob_is_err=False,
                    )
                    tvb = tvpool.tile([128, 32], F32, tag="tvb")
                    nc.gpsimd.tensor_scalar(tvb[:], ones128[:, 0:32], tv[:, s:s + 1],
                                            None, op0=OP.mult)
                    nc.gpsimd.indirect_dma_start(
                        out=B1[:],
                        out_offset=IndirectOffsetOnAxis(ap=dest_i[:, s:s + 1], axis=0),
                        in_=tvb[:], in_offset=None,
                        bounds_check=breg, oob_is_err=False,
                    )

                if s < NS - 1:
                    # next-step stationary: transpose of (sig_o * notr[s+1] * tanh_c)
                    h_m = hpool.tile([128, 512], F32, tag="hm")
                    for i in range(4):
                        csl = slice(128 * i, 128 * i + 128)
                        nc.vector.scalar_tensor_tensor(h_m[:, csl], sig_o[:, csl],
                                                       notr[:, s + 1:s + 2],
                                                       tc_t[:, csl],
                                                       op0=OP.mult, op1=OP.mult)
                    ptile = pt.tile([128, 512], F32, tag="pt")
                    for i in range(4):
                        nc.tensor.matmul(ptile[:, 128 * i:128 * i + 128],
                                         h_m[:, 128 * i:128 * i + 128], Id[:],
                                         is_transpose=True,
                                         start=(i == 0), stop=(i == 3))
                    HT = []
                    for i in range(4):
                        htt = htpool.tile([128, 128], F32R, tag="ht")
                        if i % 2 == 0:
                            nc.vector.tensor_copy(htt[:], ptile[:, 128 * i:128 * i + 128])
                        else:
                            nc.scalar.copy(htt[:], ptile[:, 128 * i:128 * i + 128])
                        HT.append(htt)
                    HT_prev = HT
                c_prev = c_new
                if s < NS - 1:
                    gA, gB = gA2, gB2

        # ================= Phase 4: bidx extraction =================
        with tc.tile_pool(name="bex", bufs=1) as bex, \
             tc.tile_pool(name="pbx", bufs=1, space="PSUM") as pbx:
            ext = bex.tile([128, 1024], F32, tag="ext")
            for bb in range(32):
                eng = nc.sync if bb % 2 == 0 else nc.scalar
                eng.dma_start(ext[:, 32 * bb:32 * bb + 32], B1r3[bb])
            bf = bex.tile([128, 32], F32, tag="bf")
            nc.vector.tensor_copy(bf[:], ext[:, 0:1024:32])
            bp = pbx.tile([32, 128], F32, tag="bp")
            nc.tensor.matmul(bp[:], bf[:], Id[:], is_transpose=True,
                             start=True, stop=True)
            bout = bex.tile([32, 128], F32, tag="bout")
            nc.vector.tensor_copy(bout[:], bp[:])
            fb = bex.tile([1, 1], F32, tag="fb")
            nc.vector.tensor_scalar(fb[:], cnt[:], 0.0, None, op0=OP.is_equal)
            fbi = bex.tile([1, 1], I32, tag="fbi")
            nc.vector.tensor_copy(fbi[:], fb[:])
            fbv = bex.tile([1, 1], F32, tag="fbv")
            nc.vector.memset(fbv[:], float(T - 1))
            nc.vector.copy_predicated(bout[0:1, 0:1], fbi[:], fbv[:])
            bi = bex.tile([32, 128], I32, tag="bi")
            nc.vector.tensor_copy(bi[:], bout[:])
            nc.sync.dma_start(bidx_o[:], bi[:])

    nc.compile()
    return nc


def _w23_65(W2):
    w = np.zeros((H1, 65), np.float32)
    w[:, 0] = W2[0]
    w[:, 32] = W2[1]
    w[:, 64] = W2[1] - W2[0]
    return w


def _bias65(b2):
    b = np.zeros((1, 65), np.float32)
    b[0, 0] = b2[0]
    b[0, 32] = b2[1]
    b[0, 64] = b2[1] - b2[0]
    return b


def make_host_inputs(x, W1, b1, W2, b2, W_ih, W_hh, b_ih, b_hh):
    """Returns per-core list of input dicts (host-side layout prep only)."""
    lanes = np.arange(128)
    lstrict = (lanes[:, None] < lanes[None, :]).astype(np.float32)      # L[q,p]=1 if q<p
    sshift = (lanes[:, None] == lanes[None, :] - 1).astype(np.float32)  # S[q,p]=1 if q==p-1
    ident = np.eye(128, dtype=np.float32)
    forced = np.zeros((128, NS), np.float32)
    forced[:, 31] = 1.0
    forced[127, 31] = 0.0
    forced[127, 30] = 1.0
    tvals = (32 * lanes[:, None] + 1 + np.arange(NS)[None, :]).astype(np.float32)
    common = {
        "wihT": np.ascontiguousarray(W_ih.T),
        "whhT": np.ascontiguousarray(W_hh.T),
        "w1T": np.ascontiguousarray(W1.T),
        "w23T": _w23_65(W2),
        "b1p": b1.reshape(2, 128, 1).astype(np.float32),
        "bias3": _bias65(b2),
        "brow": (b_ih + b_hh).reshape(1, G4).astype(np.float32),
        "lstrict": lstrict, "sshift": sshift, "ident": ident,
        "forced": forced, "tvals": tvals,
    }
    per_core = []
    for b in range(B):
        m = dict(common)
        m["xT"] = np.ascontiguousarray(x[b].T)
        per_core.append(m)
    return per_core


_NC_CACHE = {}


def kernel(x, W1, b1, W2, b2, W_ih, W_hh, b_ih, b_hh):
    from concourse.bass_utils import run_bass_kernel_spmd

    x = np.asarray(x, np.float32)
    args = [np.asarray(a, np.float32) for a in (W1, b1, W2, b2, W_ih, W_hh, b_ih, b_hh)]
    if "nc" not in _NC_CACHE:
        _NC_CACHE["nc"] = build_nc()
    nc = _NC_CACHE["nc"]
    in_maps = make_host_inputs(x, *args)
    res = run_bass_kernel_spmd(nc, in_maps, core_ids=list(range(B)))
    padded = np.stack([res.results[c]["padded"] for c in range(B)])
    bidx = np.stack([res.results[c]["bidx_o"].reshape(T) for c in range(B)])
    logits = np.stack([np.ascontiguousarray(res.results[c]["logits"].T)
                       for c in range(B)])
    return padded, bidx, logits
